# revision 1
# baseline (speedup 1.0000x reference)
"""Trainium2 Bass kernel for sliding-window multi-head attention with qk-norm.

Problem (hardcoded): B=2, S=2048, E=1024, H=16, D=64, WINDOW=512, fp32.

Sharding: heads across 8 cores (2 heads/core, all tokens), AllToAll of head
outputs, token-split out-projection (512 tokens/core).

v3 design notes:
- r_q, r_k (LN inverse-stddev), gamma-product and 1/sqrt(D) are folded into
  qc/kc at projection time, so attention softmax is one fused Exp per
  query-chunk over [128, nblocks*256] PSUM.
- r rows are exp(-0.5*ln(var+eps)) on ScalarE (Ln+Exp share one table set).
- Softmax normalization is deferred: raw (sum p*v, rowsum) ride the AllToAll
  as 65-row slots; dest cores normalize via reciprocal_approx + broadcast MMs.
- Software pipelining: the PE queue is in-order, so each loop emits the next
  unit's dense matmuls BEFORE the previous unit's stats/attnV matmuls (whose
  inputs are produced by DVE/ScalarE during the dense group). This keeps the
  PE busy and the HAM clock-gate warm.
- Out-projection is split into a2aA-half and a2aB-half accumulation so the
  A-half runs during the second collective.
"""

import sys

sys.path.insert(0, "/opt/trn_rl_repo")

import numpy as np
import ml_dtypes

import concourse.bass as bass
import concourse.mybir as mybir
import concourse.tile as tile
from concourse import bacc
from concourse.bass_utils import run_bass_kernel_spmd

F32 = mybir.dt.float32
F32R = mybir.dt.float32r
BF16 = mybir.dt.bfloat16
AF = mybir.ActivationFunctionType

B, S, E, H = 2, 2048, 1024, 16
D = E // H  # 64
WINDOW = 512
EPS = 1e-5
N_CORES = 8
HPC = H // N_CORES  # heads per core = 2
TOK = B * S  # 4096
CHUNK = 512  # token chunk for projection phase
NCHUNK = TOK // CHUNK  # 8
CPB = NCHUNK // B  # chunks per batch = 4
QCH = 256  # query chunk for attention
NQCH = S // QCH  # 8 per (batch, head)


def _blocks_for_chunk(qs):
    out = []
    for i in range(6):
        ks = qs - 512 + 128 * i
        if ks >= 0:
            out.append(ks)
    return out


def build_program():
    nc = bacc.Bacc("TRN2", target_bir_lowering=False, debug=False,
                   num_devices=N_CORES)

    # ---- dram parameters (per-core inputs; host pre-arranged layouts) ----
    xT = nc.declare_dram_parameter("xT", [128, 8, TOK], BF16, isOutput=False)
    wqkv = nc.declare_dram_parameter("wqkv", [128, 8, 384], BF16, isOutput=False)
    bqkv = nc.declare_dram_parameter("bqkv", [128, 3], F32, isOutput=False)
    wout = nc.declare_dram_parameter("wout", [128, 8, E], BF16, isOutput=False)
    bout = nc.declare_dram_parameter("bout", [128, 8], F32, isOutput=False)
    # tri[:,0,:] = T1[k,q] = (q<k); tri[:,1,:] = T2[k,q] = (k<=q)
    tri = nc.declare_dram_parameter("tri", [128, 2, 128], BF16, isOutput=False)
    selbf = nc.declare_dram_parameter("selbf", [128, 128], BF16, isOutput=False)
    sel2 = nc.declare_dram_parameter("sel2", [128, 2], BF16, isOutput=False)
    epsc = nc.declare_dram_parameter("epsc", [2, 1], F32, isOutput=False)
    expdq = nc.declare_dram_parameter("expdq", [2, 128], BF16, isOutput=False)
    expdk = nc.declare_dram_parameter("expdk", [2, 128], BF16, isOutput=False)
    seloA = nc.declare_dram_parameter("seloA", [8, 8 * 128], BF16, isOutput=False)
    seloB = nc.declare_dram_parameter("seloB", [8, 8 * 128], BF16, isOutput=False)
    identb = nc.declare_dram_parameter("identb", [128, 128], BF16, isOutput=False)
    outT = nc.declare_dram_parameter("outT", [E, 512], F32, isOutput=True)

    with tile.TileContext(nc) as tc:
        with (
            nc.allow_low_precision(reason="bf16 matmul pipeline"),
            tc.tile_pool(name="const", bufs=1) as cpool,
            tc.tile_pool(name="xp", bufs=1) as xpool,
            tc.tile_pool(name="persist", bufs=1) as ppool,
            tc.tile_pool(name="work", bufs=4) as wp,
            tc.tile_pool(name="expp", bufs=2) as epool,
            tc.tile_pool(name="rhsp", bufs=1) as rpool,
            tc.tile_pool(name="outp", bufs=2) as opool,
            tc.tile_pool(name="dram", bufs=1, space="DRAM") as dpool,
        ):
            # ---- internal dram for collectives (per hl, per column half) ----
            parts = {}
            a2as = {}
            for hl in range(2):
                for h in range(2):
                    parts[(hl, h)] = dpool.tile(
                        [N_CORES, 65, 256], BF16, name=f"part{hl}{h}")
                    a2as[(hl, h)] = dpool.tile(
                        [N_CORES, 65, 256], BF16, name=f"a2a{hl}{h}")

            # ---- constants: tiny ones first, then wqkv, then streamed x ----
            bqkv_sb = cpool.tile([128, 3], F32)
            nc.sync.dma_start(out=bqkv_sb, in_=bqkv[:, :])
            selbf_sb = cpool.tile([128, 128], BF16)
            nc.sync.dma_start(out=selbf_sb, in_=selbf[:, :])
            sel2_sb = cpool.tile([128, 2], BF16)
            nc.sync.dma_start(out=sel2_sb, in_=sel2[:, :])
            epsc_sb = cpool.tile([2, 1], F32)
            nc.sync.dma_start(out=epsc_sb, in_=epsc[:, :])
            expdq_sb = cpool.tile([2, 128], BF16)
            nc.sync.dma_start(out=expdq_sb, in_=expdq[:, :])
            expdk_sb = cpool.tile([2, 128], BF16)
            nc.sync.dma_start(out=expdk_sb, in_=expdk[:, :])
            identb_sb = cpool.tile([128, 128], BF16)
            nc.sync.dma_start(out=identb_sb, in_=identb[:, :])
            wqkv_sb = cpool.tile([128, 8, 384], BF16)
            nc.sync.dma_start(out=wqkv_sb, in_=wqkv[:, :, :])

            # x streamed in 8 per-chunk tiles (issue order = consume order)
            xc = []
            for t in range(NCHUNK):
                xc_t = xpool.tile([128, 8, CHUNK], BF16, tag=f"xc{t}",
                                  name=f"xc{t}")
                nc.sync.dma_start(
                    out=xc_t, in_=xT[:, :, t * CHUNK:(t + 1) * CHUNK])
                xc.append(xc_t)

            # attention/outproj constants ride behind the x stream
            tri_sb = cpool.tile([128, 2, 128], BF16)
            nc.sync.dma_start(out=tri_sb, in_=tri[:, :, :])
            bout_sb = cpool.tile([128, 8], F32)
            nc.sync.dma_start(out=bout_sb, in_=bout[:, :])
            seloA_sb = cpool.tile([8, 8 * 128], BF16)
            nc.sync.dma_start(out=seloA_sb, in_=seloA[:, :])
            seloB_sb = cpool.tile([8, 8 * 128], BF16)
            nc.sync.dma_start(out=seloB_sb, in_=seloB[:, :])

            # wout loaded late (not needed until phase 3)
            wout_sb = cpool.tile([128, 8, E], BF16)

            # ---- persistent per-batch tensors ----
            qc = [ppool.tile([128, S], BF16, tag=f"qc{b}", name=f"qc{b}")
                  for b in range(B)]
            kc = [ppool.tile([128, S], BF16, tag=f"kc{b}", name=f"kc{b}")
                  for b in range(B)]
            # vhat per 128-token block: [v0(64), one, pad, v1(64), one, pad]
            vhat = [ppool.tile([128, S // 128, 132], BF16, tag=f"vh{b}",
                    name=f"vh{b}") for b in range(B)]

            for b in range(B):
                nc.vector.memset(vhat[b][:, :, 64:65].bitcast(mybir.dt.uint16),
                                 0x3F80)
                nc.vector.memset(vhat[b][:, :, 130:131].bitcast(mybir.dt.uint16),
                                 0x3F80)

            # ================= Phase 1: qkv projection + qk-norm =============
            with (
                tc.tile_pool(name="psA_mm", bufs=4, space="PSUM") as psA_mm,
                tc.tile_pool(name="psA_st", bufs=2, space="PSUM") as psA_st,
                tc.tile_pool(name="psA_vr", bufs=1, space="PSUM") as psA_vr,
            ):
                def proj_mms(t):
                    xt = xc[t]
                    mms = []
                    for c3 in range(3):  # 0=q, 1=k, 2=v
                        mm = psA_mm.tile([128, CHUNK], F32, tag="mm")
                        for et in range(8):
                            nc.tensor.matmul(
                                mm[:],
                                wqkv_sb[:, et, c3 * 128:(c3 + 1) * 128],
                                xt[:, et, :],
                                start=(et == 0),
                                stop=(et == 7),
                            )
                        mms.append(mm)
                    return mms

                def proj_tail(t, mms):
                    b = t // CPB
                    ts = (t % CPB) * CHUNK
                    sq = wp.tile([128, 2 * CHUNK], BF16, tag="sq", bufs=2)
                    dqs = []
                    for c3 in range(2):
                        xsb = wp.tile([128, CHUNK], BF16, tag="xsb")
                        nc.vector.tensor_scalar_add(
                            xsb[:], mms[c3][:], bqkv_sb[:, c3:c3 + 1])
                        mu = psA_st.tile([128, CHUNK], F32, tag="st")
                        nc.tensor.matmul(mu[:], selbf_sb[:], xsb[:],
                                         start=True, stop=True)
                        dq = wp.tile([128, CHUNK], BF16, tag="dq")
                        nc.vector.tensor_sub(dq[:], xsb[:], mu[:])
                        dqs.append(dq)
                        nc.vector.tensor_mul(
                            sq[:, c3 * CHUNK:(c3 + 1) * CHUNK], dq[:], dq[:])
                    # V: biased copy (scalar), transpose via DMA xbar
                    vsb = wp.tile([128, CHUNK], BF16, tag="vsb")
                    nc.scalar.activation(vsb[:], mms[2][:], AF.Identity,
                                         bias=bqkv_sb[:, 2:3])
                    for j in range(CHUNK // 128):
                        blk = (ts + j * 128) // 128
                        vtt = wp.tile([128, 128], BF16, tag="vtt")
                        nc.sync.dma_start_transpose(
                            out=vtt[:, :], in_=vsb[:, j * 128:(j + 1) * 128])
                        dst = vhat[b][:, blk, :].rearrange(
                            "p (two dd) -> p two dd", two=2)[:, :, 0:64]
                        src = vtt[:, :].rearrange(
                            "p (two dd) -> p two dd", two=2)
                        nc.vector.tensor_copy(dst, src)
                    vr = psA_vr.tile([2, 2 * CHUNK], F32, tag="vr")
                    nc.tensor.matmul(vr[:, 0:CHUNK], sel2_sb[:],
                                     sq[:, 0:CHUNK], start=True, stop=True)
                    nc.tensor.matmul(vr[:, CHUNK:2 * CHUNK], sel2_sb[:],
                                     sq[:, CHUNK:2 * CHUNK],
                                     start=True, stop=True)
                    lnv = wp.tile([2, 2 * CHUNK], F32, tag="lnv", bufs=2)
                    nc.scalar.activation(lnv[:], vr[:], AF.Ln,
                                         bias=epsc_sb[:, 0:1])
                    rr = wp.tile([2, 2 * CHUNK], BF16, tag="rr", bufs=2)
                    nc.scalar.activation(rr[:], lnv[:], AF.Exp, scale=-0.5)
                    for c3 in range(2):
                        rbc = psA_st.tile([128, CHUNK], F32, tag="st")
                        nc.tensor.matmul(
                            rbc[:],
                            expdq_sb[:] if c3 == 0 else expdk_sb[:],
                            rr[:, c3 * CHUNK:(c3 + 1) * CHUNK],
                            start=True, stop=True)
                        dst = qc[b] if c3 == 0 else kc[b]
                        nc.vector.tensor_mul(dst[:, ts:ts + CHUNK],
                                             dqs[c3][:], rbc[:])

                prev = None
                for t in range(NCHUNK):
                    mms = proj_mms(t)
                    if prev is not None:
                        proj_tail(prev[0], prev[1])
                    prev = (t, mms)
                proj_tail(prev[0], prev[1])

            # wout arrives while attention runs
            nc.sync.dma_start(out=wout_sb, in_=wout[:, :, :])

            # ================= Phase 2: attention ============================
            with (
                tc.tile_pool(name="psB_sc", bufs=2, space="PSUM") as psB_sc,
                tc.tile_pool(name="psB_at", bufs=2, space="PSUM") as psB_at,
            ):
                def attn_head(u):
                    hl, b, ch = u
                    qs = ch * QCH
                    blocks = _blocks_for_chunk(qs)
                    nb = len(blocks)
                    r0, r1 = 64 * hl, 64 * hl + 64
                    sc = psB_sc.tile([128, 6 * QCH], F32, tag="sc")
                    for bi, ks in enumerate(blocks):
                        nc.tensor.matmul(
                            sc[:, bi * QCH:(bi + 1) * QCH],
                            kc[b][r0:r1, ks:ks + 128],
                            qc[b][r0:r1, qs:qs + QCH],
                            start=True, stop=True)
                    ex = epool.tile([128, 6 * QCH], BF16, tag="ex")
                    nc.scalar.activation(ex[:, 0:nb * QCH],
                                         sc[:, 0:nb * QCH], AF.Exp)
                    for bi, ks in enumerate(blocks):
                        off = ks - qs
                        e0 = ex[:, bi * QCH:bi * QCH + 128]
                        e1 = ex[:, bi * QCH + 128:(bi + 1) * QCH]
                        if off == -512:
                            nc.vector.tensor_mul(e0, e0, tri_sb[:, 0, :])
                            nc.vector.memset(e1, 0.0)
                        elif off == -384:
                            nc.vector.tensor_mul(e1, e1, tri_sb[:, 0, :])
                        elif off == 0:
                            nc.vector.tensor_mul(e0, e0, tri_sb[:, 1, :])
                        elif off == 128:
                            nc.vector.memset(e0, 0.0)
                            nc.vector.tensor_mul(e1, e1, tri_sb[:, 1, :])
                    return (u, blocks, ex)

                def attn_tail(state):
                    (hl, b, ch), blocks, ex = state
                    qs = ch * QCH
                    nb = len(blocks)
                    part = parts[(hl, ch % 2)]
                    at = psB_at.tile([65, QCH], F32, tag="at")
                    for bi, ks in enumerate(blocks):
                        nc.tensor.matmul(
                            at[:],
                            vhat[b][:, ks // 128, 66 * hl:66 * hl + 65],
                            ex[:, bi * QCH:(bi + 1) * QCH],
                            start=(bi == 0),
                            stop=(bi == nb - 1))
                    hot = wp.tile([65, QCH], BF16, tag="hot")
                    nc.vector.tensor_copy(hot[:], at[:])
                    nc.sync.dma_start(
                        out=part[b * 4 + qs // 512, :, :],
                        in_=hot[:],
                    )

                def a2a_send(hl, h):
                    nc.gpsimd.collective_compute(
                        "AllToAll",
                        mybir.AluOpType.bypass,
                        replica_groups=[list(range(N_CORES))],
                        ins=[parts[(hl, h)].opt()],
                        outs=[a2as[(hl, h)].opt()],
                    )

                units = []
                for hl in range(2):
                    for b in range(B):
                        chs = (list(range(NQCH)) if b == 0
                               else [0, 2, 4, 6, 1, 3, 5, 7])
                        units += [(hl, b, ch) for ch in chs]
                prev = None
                for u in units:
                    st = attn_head(u)
                    if prev is not None:
                        attn_tail(prev)
                        if prev[0] == (0, 1, 6):
                            a2a_send(0, 0)
                        elif prev[0] == (0, 1, 7):
                            a2a_send(0, 1)
                        elif prev[0] == (1, 1, 6):
                            a2a_send(1, 0)
                    prev = st
                attn_tail(prev)
                a2a_send(1, 1)

            # ================= Phase 3: out projection =======================
            with (
                tc.tile_pool(name="psC_mm", bufs=2, space="PSUM") as psC_mm,
                tc.tile_pool(name="psC_bc", bufs=2, space="PSUM") as psC_bc,
            ):
                rsh = {}
                for half in ("A", "B"):
                    rsh[half] = dict(
                        rs=rpool.tile([8, 512], BF16, tag=f"rs{half}",
                                      name=f"rs{half}"),
                        rsf=rpool.tile([8, 512], F32, tag=f"rsf{half}",
                                       name=f"rsf{half}"),
                        rc=rpool.tile([8, 512], F32, tag=f"rc{half}",
                                      name=f"rc{half}"),
                        scr=rpool.tile([8, 512], F32, tag=f"scr{half}",
                                       name=f"scr{half}"),
                        rcr=rpool.tile([8, 512], BF16, tag=f"rcr{half}",
                                       name=f"rcr{half}"),
                    )
                rhs = []
                for ht in range(8):
                    rt = rpool.tile([128, 512], BF16, tag=f"rhs{ht}",
                                    name=f"rhs{ht}")
                    for h in range(2):
                        nc.sync.dma_start(
                            out=rt[0:64, h * 256:(h + 1) * 256],
                            in_=a2as[(0, h)][ht, 0:64, :])
                        nc.sync.dma_start(
                            out=rsh["A"]["rs"][ht:ht + 1,
                                               h * 256:(h + 1) * 256],
                            in_=a2as[(0, h)][ht, 64:65, :])
                    rhs.append(rt)

                def norm_half(half, selo_sb, p0, p1):
                    hh = rsh[half]
                    nc.vector.tensor_copy(hh["rsf"][:], hh["rs"][:])
                    nc.vector.reciprocal_approx_accurate(
                        hh["rc"][:], hh["rsf"][:], hh["scr"][:])
                    nc.vector.tensor_copy(hh["rcr"][:], hh["rc"][:])
                    for ht in range(8):
                        bc = psC_bc.tile([128, 512], F32, tag="bc")
                        nc.tensor.matmul(bc[:, :],
                                         selo_sb[:, ht * 128:(ht + 1) * 128],
                                         hh["rcr"][:], start=True, stop=True)
                        bcsb = wp.tile([128, 512], BF16, tag="bcsb")
                        nc.scalar.copy(bcsb[p0:p1, :], bc[p0:p1, :])
                        nc.vector.tensor_mul(rhs[ht][p0:p1, :],
                                             rhs[ht][p0:p1, :],
                                             bcsb[p0:p1, :])

                # A-half normalization overlaps the a2aB wait
                norm_half("A", seloA_sb, 0, 64)
                for ht in range(8):
                    for h in range(2):
                        nc.sync.dma_start(
                            out=rhs[ht][64:128, h * 256:(h + 1) * 256],
                            in_=a2as[(1, h)][ht, 0:64, :])
                        nc.sync.dma_start(
                            out=rsh["B"]["rs"][ht:ht + 1,
                                               h * 256:(h + 1) * 256],
                            in_=a2as[(1, h)][ht, 64:65, :])
                norm_half("B", seloB_sb, 64, 128)
                for ot in range(8):
                    mm = psC_mm.tile([128, 512], F32, tag="mm")
                    for ht in range(8):
                        nc.tensor.matmul(
                            mm[:],
                            wout_sb[:, ht, ot * 128:(ot + 1) * 128],
                            rhs[ht][:],
                            start=(ht == 0), stop=(ht == 7))
                    osb = opool.tile([128, 512], F32, tag="osb")
                    nc.scalar.activation(osb[:], mm[:], AF.Identity,
                                         bias=bout_sb[:, ot:ot + 1])
                    nc.sync.dma_start(out=outT[ot * 128:(ot + 1) * 128, :],
                                      in_=osb[:])

    nc.compile()
    return nc


def _make_host_inputs(x, W_qkv, b_qkv, q_gamma, q_beta, k_gamma, k_beta,
                      W_out, b_out):
    assert np.allclose(q_beta, 0.0) and np.allclose(k_beta, 0.0), (
        "kernel only supports beta == 0 qk-norm")
    gp = (np.asarray(q_gamma) * np.asarray(k_gamma)).astype(np.float32)  # [64]

    bf = ml_dtypes.bfloat16
    xTf = np.transpose(np.asarray(x, np.float32), (2, 0, 1)).reshape(E, TOK)
    xTm = np.ascontiguousarray(
        xTf.reshape(8, 128, TOK).transpose(1, 0, 2)).astype(bf)  # [128,8,TOK]

    W3 = np.asarray(W_qkv, np.float32).reshape(E, 3, H, D)
    b3 = np.asarray(b_qkv, np.float32).reshape(3, H, D)

    kj = np.arange(128)[:, None]
    qi = np.arange(128)[None, :]
    trim = np.zeros((128, 2, 128), np.float32)
    trim[:, 0, :] = (qi < kj).astype(np.float32)   # T1
    trim[:, 1, :] = (kj <= qi).astype(np.float32)  # T2

    selm = np.zeros((128, 128), np.float32)
    for j in range(128):
        selm[j, (j // 64) * 64:(j // 64) * 64 + 64] = 1.0 / 64.0
    sel2m = np.zeros((128, 2), np.float32)
    sel2m[0:64, 0] = 1.0 / 64.0
    sel2m[64:128, 1] = 1.0 / 64.0
    expdqm = np.zeros((2, 128), np.float32)
    expdqm[0, 0:64] = 1.0
    expdqm[1, 64:128] = 1.0
    # 1/sqrt(D) folded here
    expdkm = np.zeros((2, 128), np.float32)
    expdkm[0, 0:64] = gp / 8.0
    expdkm[1, 64:128] = gp / 8.0
    seloAm = np.zeros((8, 8 * 128), np.float32)
    seloBm = np.zeros((8, 8 * 128), np.float32)
    for ht in range(8):
        seloAm[ht, ht * 128:ht * 128 + 64] = 1.0
        seloBm[ht, ht * 128 + 64:(ht + 1) * 128] = 1.0
    identm = np.eye(128, dtype=np.float32)
    woutm = np.ascontiguousarray(
        np.asarray(W_out, np.float32).reshape(8, 128, E).transpose(1, 0, 2)
    ).astype(bf)
    boutm = np.ascontiguousarray(
        np.asarray(b_out, np.float32).reshape(8, 128).T)  # [128, 8]

    in_maps = []
    for c in range(N_CORES):
        hsl = slice(HPC * c, HPC * (c + 1))
        wq = W3[:, :, hsl, :].reshape(E, 3 * HPC * D)
        wqm = np.ascontiguousarray(
            wq.reshape(8, 128, 384).transpose(1, 0, 2)).astype(bf)
        bq = np.ascontiguousarray(
            b3[:, hsl, :].reshape(3, 128).T.astype(np.float32))  # [128, 3]
        in_maps.append({
            "xT": xTm,
            "wqkv": wqm,
            "bqkv": bq,
            "wout": woutm,
            "bout": boutm,
            "tri": trim.astype(bf),
            "selbf": selm.astype(bf),
            "sel2": sel2m.astype(bf),
            "epsc": np.full((2, 1), EPS, np.float32),
            "expdq": expdqm.astype(bf),
            "expdk": expdkm.astype(bf),
            "seloA": seloAm.astype(bf),
            "seloB": seloBm.astype(bf),
            "identb": identm.astype(bf),
        })
    return in_maps


_CACHED = {}


def _get_program():
    if "nc" not in _CACHED:
        _CACHED["nc"] = build_program()
    return _CACHED["nc"]


def kernel(x, W_qkv, b_qkv, q_gamma, q_beta, k_gamma, k_beta, W_out, b_out,
           _trace=False, **trace_kwargs):
    in_maps = _make_host_inputs(
        x, W_qkv, b_qkv, q_gamma, q_beta, k_gamma, k_beta, W_out, b_out)
    nc = _get_program()
    res = run_bass_kernel_spmd(nc, in_maps, list(range(N_CORES)),
                               trace=_trace, **trace_kwargs)
    outTs = [res.results[c]["outT"] for c in range(N_CORES)]
    full = np.concatenate(outTs, axis=1)  # [E, TOK]
    out = full.reshape(E, B, S).transpose(1, 2, 0)
    if _trace:
        kernel.last_results = res
    return np.ascontiguousarray(out)


if __name__ == "__main__":
    import reference

    inputs = {k: np.asarray(v) for k, v in reference.setup_inputs().items()}
    expected = np.asarray(reference.reference(**inputs))
    actual = kernel(**inputs)
    err = np.abs(actual - expected)
    rel = np.linalg.norm(actual - expected) / np.linalg.norm(expected)
    print("max abs err:", err.max(), "rel fro err:", rel)

